# revision 2
# baseline (speedup 1.0000x reference)
"""Trainium2 Bass kernel: 2-layer LSTM decoder, gate-sharded across 8 cores.

v3: one joint AllGather per step (the collective core serializes ccs at
~8.5us each, so two per step was the v2 bottleneck).

Super-step s:
  PE: L0(s) -> L1(s-1) -> FC(s-4) -> X0(s+5)
  AG(s) gathers [h0(s) | h1(s-2)] (h1 half staged during step s-1).
  Landing DMAs (transposing, DRAM->SBUF) own the sync queue; staging DMAs
  ride the scalar queue; the collective doorbell + FC output DMAs ride
  gpsimd. fp16 matmuls, fp32 PSUM/state.
"""

import numpy as np

import concourse.bass as bass
import concourse.mybir as mybir
from concourse import bacc
from concourse.bass_utils import run_bass_kernel_spmd
from concourse.masks import make_identity
from concourse.tile import TileContext

P = 128
NCORES = 8
B, T, F, E, H, L, V = 128, 32, 2048, 512, 1024, 2, 10000
G = 4 * H
TB = T * B
HS = H // NCORES           # 128
GS = 4 * HS                # 512
VS = V // NCORES           # 1250
KF, KE, KH = F // P, E // P, H // P
LAG1 = 1                   # L1 step lag
LAGF = 4                   # FC step lag
X0AHEAD = 5                # X0 precompute lookahead inside the loop
F16 = mybir.dt.float16
F32 = mybir.dt.float32
RG = [list(range(NCORES))]

_cache = {}


def _build_nc():
    nc = bacc.Bacc("TRN2", target_bir_lowering=False, debug=False,
                   enable_asserts=False, num_devices=NCORES)

    def din(name, shape, dt=F16):
        return nc.dram_tensor(name, shape, dt, kind="ExternalInput").ap()

    featT = din("featT", [F, B])
    embT = din("embT", [E, TB])
    init_hwT = din("init_hwT", [F, 2 * HS])
    init_cwT = din("init_cwT", [F, 2 * HS])
    init_hb = din("init_hb", [1, 2 * HS])
    init_cb = din("init_cb", [1, 2 * HS])
    wih0T = din("wih0T", [E, GS])
    whh0T = din("whh0T", [H, GS])
    wih1T = din("wih1T", [H, GS])
    whh1T = din("whh1T", [H, GS])
    bsum0 = din("bsum0", [1, GS])
    bsum1 = din("bsum1", [1, GS])
    fcwT = din("fcwT", [H, VS])
    fcb = din("fcb", [P, VS], F32)

    out = nc.dram_tensor("out", [TB, VS], F32, kind="ExternalOutput").ap()

    featT_v = featT.rearrange("(k p) b -> p k b", p=P)
    embT_v = embT.rearrange("(k p) t -> p k t", p=P)
    ihw_v = init_hwT.rearrange("(k p) n -> p k n", p=P)
    icw_v = init_cwT.rearrange("(k p) n -> p k n", p=P)
    wih0_v = wih0T.rearrange("(k p) g -> p k g", p=P)
    whh0_v = whh0T.rearrange("(k p) g -> p k g", p=P)
    wih1_v = wih1T.rearrange("(k p) g -> p k g", p=P)
    whh1_v = whh1T.rearrange("(k p) g -> p k g", p=P)
    fcw_v = fcwT.rearrange("(k p) v -> p k v", p=P)

    SIG = mybir.ActivationFunctionType.Sigmoid
    TANH = mybir.ActivationFunctionType.Tanh

    with TileContext(nc) as tc:
        with tc.tile_pool(name="const", bufs=1) as constp, \
             tc.tile_pool(name="resw", bufs=1) as resw, \
             tc.tile_pool(name="x0", bufs=1) as x0p, \
             tc.tile_pool(name="state", bufs=1) as statep, \
             tc.tile_pool(name="h0t", bufs=3) as h0tp, \
             tc.tile_pool(name="yst", bufs=6) as ystp, \
             tc.tile_pool(name="dram", bufs=4, space="DRAM") as drp:

            id128 = constp.tile([P, P], F16)
            make_identity(nc, id128)
            ones1 = constp.tile([1, P], F16)
            nc.gpsimd.memset(ones1, 1.0)

            whh0_s = resw.tile([P, KH, GS], F16)
            nc.sync.dma_start(whh0_s, whh0_v)
            wih1_s = resw.tile([P, KH, GS], F16)
            nc.sync.dma_start(wih1_s, wih1_v)
            whh1_s = resw.tile([P, KH, GS], F16)
            nc.sync.dma_start(whh1_s, whh1_v)
            bsum1_s = resw.tile([1, GS], F16)
            nc.sync.dma_start(bsum1_s, bsum1)
            fcw_s = resw.tile([P, KH, VS], F16)
            nc.sync.dma_start(fcw_s, fcw_v)
            fcb_s = resw.tile([P, VS], F32)
            nc.sync.dma_start(fcb_s, fcb)
            embT_s = resw.tile([P, KE, TB], F16)
            nc.sync.dma_start(embT_s, embT_v)
            wih0_s = resw.tile([P, KE, GS], F16)
            nc.sync.dma_start(wih0_s, wih0_v)
            bsum0_s = resw.tile([1, GS], F16)
            nc.sync.dma_start(bsum0_s, bsum0)

            X0_s = x0p.tile([P, T, GS], F16)
            c0_s = statep.tile([P, HS], F32)
            c1_s = statep.tile([P, HS], F32)

            # ------------- Phase B: init h/c ------------------------------
            with tc.tile_pool(name="initw", bufs=1) as initw, \
                 tc.tile_pool(name="initps", bufs=1, space="PSUM") as initps:
                featT_s = initw.tile([P, KF, B], F16, tag="ft")
                nc.sync.dma_start(featT_s, featT_v)
                ihw_s = initw.tile([P, KF, 2 * HS], F16, tag="ihw")
                nc.sync.dma_start(ihw_s, ihw_v)
                icw_s = initw.tile([P, KF, 2 * HS], F16, tag="icw")
                nc.sync.dma_start(icw_s, icw_v)
                ihb_s = initw.tile([1, 2 * HS], F16, tag="ihb")
                nc.sync.dma_start(ihb_s, init_hb)
                icb_s = initw.tile([1, 2 * HS], F16, tag="icb")
                nc.sync.dma_start(icb_s, init_cb)

                ps_h = initps.tile([P, 2 * HS], F32, tag="psh")
                for k in range(KF):
                    nc.tensor.matmul(ps_h, featT_s[:, k, :], ihw_s[:, k, :],
                                     start=(k == 0), stop=False)
                nc.tensor.matmul(ps_h, ones1, ihb_s, start=False, stop=True)
                ps_c = initps.tile([P, 2 * HS], F32, tag="psc")
                for k in range(KF):
                    nc.tensor.matmul(ps_c, featT_s[:, k, :], icw_s[:, k, :],
                                     start=(k == 0), stop=False)
                nc.tensor.matmul(ps_c, ones1, icb_s, start=False, stop=True)

                nc.vector.tensor_copy(c0_s, ps_c[:, 0:HS])
                nc.vector.tensor_copy(c1_s, ps_c[:, HS:2 * HS])
                h_init = initw.tile([P, 2 * HS], F16, tag="hinit")
                nc.vector.tensor_copy(h_init, ps_h)

                ib_in = drp.tile([P, 2 * HS], F16, tag="ibin")
                nc.scalar.dma_start(ib_in, h_init)
                ib_out = drp.tile([NCORES * P, 2 * HS], F16, tag="ibout")
                nc.gpsimd.collective_compute(
                    "AllGather", mybir.AluOpType.bypass, replica_groups=RG,
                    ins=[ib_in[:]], outs=[ib_out[:]],
                )
                h0T_cur = h0tp.tile([P, KH, B], F16, tag="h0T")
                nc.sync.dma_start_transpose(h0T_cur, ib_out[:, 0:HS])
                ysT_init = ystp.tile([P, KH, B], F16, tag="ysT")
                nc.sync.dma_start_transpose(ysT_init, ib_out[:, HS:2 * HS])

            # ------------- Phase D: recurrence + FC + inline X0 -----------
            with tc.tile_pool(name="act", bufs=3) as actp, \
                 tc.tile_pool(name="hstage", bufs=3) as hstage, \
                 tc.tile_pool(name="fco", bufs=4) as fcop, \
                 tc.tile_pool(name="g0ps", bufs=2, space="PSUM") as g0ps, \
                 tc.tile_pool(name="g1ps", bufs=2, space="PSUM") as g1ps, \
                 tc.tile_pool(name="fcps", bufs=2, space="PSUM") as fcps, \
                 tc.tile_pool(name="cps", bufs=1, space="PSUM") as cps, \
                 tc.tile_pool(name="warmps", bufs=1, space="PSUM") as warmps:

                def x0_compute(t):
                    ps = cps.tile([P, GS], F32, tag="cps")
                    for k in range(KE):
                        nc.tensor.matmul(
                            ps, embT_s[:, k, t * B:(t + 1) * B],
                            wih0_s[:, k, :], start=(k == 0), stop=False)
                    nc.tensor.matmul(ps, ones1, bsum0_s,
                                     start=False, stop=True)
                    nc.vector.tensor_copy(X0_s[:, t, :], ps)

                def lstm_tail(ps, c_s, sigtag, htag):
                    sig = actp.tile([P, 3 * HS], F32, tag=f"sig{sigtag}")
                    nc.scalar.activation(sig, ps[:, 0:3 * HS], SIG)
                    tg = actp.tile([P, HS], F32, tag=f"tg{sigtag}")
                    nc.scalar.activation(tg, ps[:, 3 * HS:4 * HS], TANH)
                    nc.vector.tensor_mul(c_s, sig[:, HS:2 * HS], c_s)
                    nc.vector.tensor_mul(tg, sig[:, 0:HS], tg)
                    nc.vector.tensor_add(c_s, c_s, tg)
                    tcn = actp.tile([P, HS], F32, tag=f"tc{sigtag}")
                    nc.scalar.activation(tcn, c_s, TANH)
                    h_sb = hstage.tile([P, HS], F16, tag=htag)
                    nc.vector.tensor_mul(h_sb, sig[:, 2 * HS:3 * HS], tcn)
                    return h_sb

                h0T_hist = {-1: h0T_cur}
                ysT_hist = {-1: ysT_init}

                # X0 prologue
                for t in range(min(X0AHEAD, T)):
                    x0_compute(t)

                # Super-step s: L1(s-1) first, then L0(s); ONE joint
                # AllGather carries [h0(s) | h1(s-1)] so only a single
                # collective is ever in flight (a queued second cc delays
                # ncfw's completion processing of the first by ~10us).
                for s in range(T + LAGF + 1):
                    t0, t1, t2 = s, s - LAG1, s - LAGF
                    tx = s + X0AHEAD
                    b_in = b_out = None
                    if t0 < T or (0 <= t1 < T):
                        b_in = drp.tile([P, 2 * HS], F16, tag="bin",
                                        name="bin")
                        b_out = drp.tile([NCORES * P, 2 * HS], F16,
                                         tag="bout", name="bout")

                    if 0 <= t1 < T:
                        ps1 = g1ps.tile([P, GS], F32, tag="g1")
                        h0T_t1 = h0T_hist[t1]
                        for k in range(KH):
                            nc.tensor.matmul(ps1, h0T_t1[:, k, :],
                                             wih1_s[:, k, :],
                                             start=(k == 0), stop=False)
                        ysT_prev = ysT_hist[t1 - 1]
                        for k in range(KH):
                            nc.tensor.matmul(ps1, ysT_prev[:, k, :],
                                             whh1_s[:, k, :],
                                             start=False, stop=False)
                        nc.tensor.matmul(ps1, ones1, bsum1_s,
                                         start=False, stop=True)
                        h1_sb = lstm_tail(ps1, c1_s, 1, "h1sb")
                        nc.scalar.dma_start(b_in[:, HS:2 * HS], h1_sb)
                        h0T_hist.pop(t1 - 1, None)
                        ysT_hist.pop(t1 - 4, None)

                    if t0 < T:
                        ps0 = g0ps.tile([P, GS], F32, tag="g0")
                        nc.tensor.matmul(ps0, id128, X0_s[:, t0, :],
                                         start=True, stop=False)
                        h0T_prev = h0T_hist[t0 - 1]
                        for k in range(KH):
                            nc.tensor.matmul(ps0, h0T_prev[:, k, :],
                                             whh0_s[:, k, :],
                                             start=False, stop=(k == KH - 1))
                        h0_sb = lstm_tail(ps0, c0_s, 0, "h0sb")
                        nc.scalar.dma_start(b_in[:, 0:HS], h0_sb)

                    if b_in is not None:
                        nc.gpsimd.collective_compute(
                            "AllGather", mybir.AluOpType.bypass,
                            replica_groups=RG,
                            ins=[b_in[:]], outs=[b_out[:]],
                        )

                    if 0 <= t2 < T:
                        yt = ysT_hist[t2]
                        for ccol, w in ((0, 512), (512, 512), (1024, VS - 1024)):
                            psf = fcps.tile([P, 512], F32, tag="fc")
                            for k in range(KH):
                                nc.tensor.matmul(psf[:, :w], yt[:, k, :],
                                                 fcw_s[:, k, ccol:ccol + w],
                                                 start=(k == 0),
                                                 stop=(k == KH - 1))
                            ot = fcop.tile([P, 512], F32, tag="fco")
                            nc.vector.tensor_add(ot[:, :w], psf[:, :w],
                                                 fcb_s[:, ccol:ccol + w])
                            nc.sync.dma_start(
                                out[t2 * B:(t2 + 1) * B, ccol:ccol + w],
                                ot[:, :w])

                    if tx < T:
                        x0_compute(tx)

                    # land the joint gather: h0T on sync, ysT on scalar so
                    # the two transposing DMAs run in parallel
                    if b_out is not None and t0 < T:
                        hT = h0tp.tile([P, KH, B], F16, tag="h0T")
                        nc.sync.dma_start_transpose(hT, b_out[:, 0:HS])
                        h0T_hist[t0] = hT
                    if b_out is not None and 0 <= t1 < T:
                        yT = ystp.tile([P, KH, B], F16, tag="ysT")
                        nc.sync.dma_start_transpose(yT, b_out[:, HS:2 * HS])
                        ysT_hist[t1] = yT

    nc.finalize()
    return nc


def _get_compiled():
    if "nc" not in _cache:
        _cache["nc"] = _build_nc()
    return _cache["nc"]


def _prep_inputs(features, captions, embed_table, init_h_w, init_h_b,
                 init_c_w, init_c_b, w_ih0, w_hh0, b_ih0, b_hh0,
                 w_ih1, w_hh1, b_ih1, b_hh1, fc_w, fc_b):
    f32 = np.float32
    qorder = [0, 1, 3, 2]  # torch gate order i,f,g,o -> ours i,f,o,g

    def gate_cols(wmat, c):
        w = np.asarray(wmat, f32).reshape(4, H, -1)
        sl = w[:, c * HS:(c + 1) * HS, :][qorder]
        return np.ascontiguousarray(
            sl.reshape(4 * HS, -1).T).astype(np.float16)

    def gate_bias(b1, b2, c):
        sb = np.asarray(b1, f32) + np.asarray(b2, f32)
        sl = sb.reshape(4, H)[:, c * HS:(c + 1) * HS][qorder]
        return np.ascontiguousarray(sl.reshape(1, 4 * HS)).astype(np.float16)

    def init_wT(wmat, c):
        w = np.asarray(wmat, f32).reshape(H, L, F)
        blk = np.concatenate(
            [w[c * HS:(c + 1) * HS, 0, :], w[c * HS:(c + 1) * HS, 1, :]], 0)
        return np.ascontiguousarray(blk.T).astype(np.float16)

    def init_b(bvec, c):
        b = np.asarray(bvec, f32).reshape(H, L)
        blk = np.concatenate(
            [b[c * HS:(c + 1) * HS, 0], b[c * HS:(c + 1) * HS, 1]])
        return np.ascontiguousarray(blk.reshape(1, 2 * HS)).astype(np.float16)

    captions = np.asarray(captions)
    tbl = np.asarray(embed_table, f32)
    idx = captions.T.reshape(TB).astype(np.int64)
    emb = tbl[idx]
    emb[idx == 0] = 0.0
    embT_np = np.ascontiguousarray(emb.T).astype(np.float16)
    featT_np = np.ascontiguousarray(
        np.asarray(features, f32).T).astype(np.float16)

    fcw = np.asarray(fc_w, f32)
    fcb_np = np.asarray(fc_b, f32)

    in_maps = []
    for c in range(NCORES):
        vsl = slice(c * VS, (c + 1) * VS)
        in_maps.append({
            "featT": featT_np,
            "embT": embT_np,
            "init_hwT": init_wT(init_h_w, c),
            "init_cwT": init_wT(init_c_w, c),
            "init_hb": init_b(init_h_b, c),
            "init_cb": init_b(init_c_b, c),
            "wih0T": gate_cols(w_ih0, c),
            "whh0T": gate_cols(w_hh0, c),
            "wih1T": gate_cols(w_ih1, c),
            "whh1T": gate_cols(w_hh1, c),
            "bsum0": gate_bias(b_ih0, b_hh0, c),
            "bsum1": gate_bias(b_ih1, b_hh1, c),
            "fcwT": np.ascontiguousarray(fcw[vsl].T).astype(np.float16),
            "fcb": np.ascontiguousarray(
                np.broadcast_to(fcb_np[vsl], (P, VS))).astype(f32),
        })
    return in_maps


last_results = None


def kernel(**inputs) -> np.ndarray:
    global last_results
    nc = _get_compiled()
    in_maps = _prep_inputs(**inputs)
    res = run_bass_kernel_spmd(nc, in_maps, core_ids=list(range(NCORES)))
    last_results = res
    parts = [res.results[c]["out"].reshape(T, B, VS) for c in range(NCORES)]
    return np.concatenate(parts, axis=2)


# revision 3
# speedup vs baseline: 1.0485x; 1.0485x over previous
"""Trainium2 Bass kernel: 2-layer LSTM decoder, gate-sharded across 8 cores.

v3: one joint AllGather per step (the collective core serializes ccs at
~8.5us each, so two per step was the v2 bottleneck).

Super-step s:
  PE: L0(s) -> L1(s-1) -> FC(s-4) -> X0(s+5)
  AG(s) gathers [h0(s) | h1(s-2)] (h1 half staged during step s-1).
  Landing DMAs (transposing, DRAM->SBUF) own the sync queue; staging DMAs
  ride the scalar queue; the collective doorbell + FC output DMAs ride
  gpsimd. fp16 matmuls, fp32 PSUM/state.
"""

import numpy as np

import concourse.bass as bass
import concourse.mybir as mybir
from concourse import bacc
from concourse.bass_utils import run_bass_kernel_spmd
from concourse.masks import make_identity
from concourse.tile import TileContext

P = 128
NCORES = 8
B, T, F, E, H, L, V = 128, 32, 2048, 512, 1024, 2, 10000
G = 4 * H
TB = T * B
HS = H // NCORES           # 128
GS = 4 * HS                # 512
VS = V // NCORES           # 1250
KF, KE, KH = F // P, E // P, H // P
LAG1 = 1                   # L1 step lag
LAGF = 4                   # FC step lag
X0AHEAD = 5                # X0 precompute lookahead inside the loop
F16 = mybir.dt.float16
F32 = mybir.dt.float32
RG = [list(range(NCORES))]

_cache = {}


def _build_nc():
    nc = bacc.Bacc("TRN2", target_bir_lowering=False, debug=False,
                   enable_asserts=False, num_devices=NCORES)

    def din(name, shape, dt=F16):
        return nc.dram_tensor(name, shape, dt, kind="ExternalInput").ap()

    featT = din("featT", [F, B])
    embT = din("embT", [E, TB])
    init_hwT = din("init_hwT", [F, 2 * HS])
    init_cwT = din("init_cwT", [F, 2 * HS])
    init_hb = din("init_hb", [1, 2 * HS])
    init_cb = din("init_cb", [1, 2 * HS])
    wih0T = din("wih0T", [E, GS])
    whh0T = din("whh0T", [H, GS])
    wih1T = din("wih1T", [H, GS])
    whh1T = din("whh1T", [H, GS])
    bsum0 = din("bsum0", [1, GS])
    bsum1 = din("bsum1", [1, GS])
    fcwT = din("fcwT", [H, VS])
    fcb = din("fcb", [P, VS], F32)

    out = nc.dram_tensor("out", [TB, VS], F32, kind="ExternalOutput").ap()

    featT_v = featT.rearrange("(k p) b -> p k b", p=P)
    embT_v = embT.rearrange("(k p) t -> p k t", p=P)
    ihw_v = init_hwT.rearrange("(k p) n -> p k n", p=P)
    icw_v = init_cwT.rearrange("(k p) n -> p k n", p=P)
    wih0_v = wih0T.rearrange("(k p) g -> p k g", p=P)
    whh0_v = whh0T.rearrange("(k p) g -> p k g", p=P)
    wih1_v = wih1T.rearrange("(k p) g -> p k g", p=P)
    whh1_v = whh1T.rearrange("(k p) g -> p k g", p=P)
    fcw_v = fcwT.rearrange("(k p) v -> p k v", p=P)

    SIG = mybir.ActivationFunctionType.Sigmoid
    TANH = mybir.ActivationFunctionType.Tanh

    with TileContext(nc) as tc:
        with tc.tile_pool(name="const", bufs=1) as constp, \
             tc.tile_pool(name="resw", bufs=1) as resw, \
             tc.tile_pool(name="x0", bufs=1) as x0p, \
             tc.tile_pool(name="state", bufs=1) as statep, \
             tc.tile_pool(name="h0t", bufs=3) as h0tp, \
             tc.tile_pool(name="yst", bufs=6) as ystp, \
             tc.tile_pool(name="dram", bufs=4, space="DRAM") as drp:

            id128 = constp.tile([P, P], F16)
            make_identity(nc, id128)
            ones1 = constp.tile([1, P], F16)
            nc.gpsimd.memset(ones1, 1.0)

            whh0_s = resw.tile([P, KH, GS], F16)
            nc.sync.dma_start(whh0_s, whh0_v)
            wih1_s = resw.tile([P, KH, GS], F16)
            nc.sync.dma_start(wih1_s, wih1_v)
            whh1_s = resw.tile([P, KH, GS], F16)
            nc.sync.dma_start(whh1_s, whh1_v)
            bsum1_s = resw.tile([1, GS], F16)
            nc.sync.dma_start(bsum1_s, bsum1)
            fcw_s = resw.tile([P, KH, VS], F16)
            nc.sync.dma_start(fcw_s, fcw_v)
            fcb_s = resw.tile([P, VS], F32)
            nc.sync.dma_start(fcb_s, fcb)
            embT_s = resw.tile([P, KE, TB], F16)
            nc.sync.dma_start(embT_s, embT_v)
            wih0_s = resw.tile([P, KE, GS], F16)
            nc.sync.dma_start(wih0_s, wih0_v)
            bsum0_s = resw.tile([1, GS], F16)
            nc.sync.dma_start(bsum0_s, bsum0)

            X0_s = x0p.tile([P, T, GS], F16)
            c0_s = statep.tile([P, HS], F32)
            c1_s = statep.tile([P, HS], F32)

            # ------------- Phase B: init h/c ------------------------------
            with tc.tile_pool(name="initw", bufs=1) as initw, \
                 tc.tile_pool(name="initps", bufs=1, space="PSUM") as initps:
                featT_s = initw.tile([P, KF, B], F16, tag="ft")
                nc.sync.dma_start(featT_s, featT_v)
                ihw_s = initw.tile([P, KF, 2 * HS], F16, tag="ihw")
                nc.sync.dma_start(ihw_s, ihw_v)
                icw_s = initw.tile([P, KF, 2 * HS], F16, tag="icw")
                nc.sync.dma_start(icw_s, icw_v)
                ihb_s = initw.tile([1, 2 * HS], F16, tag="ihb")
                nc.sync.dma_start(ihb_s, init_hb)
                icb_s = initw.tile([1, 2 * HS], F16, tag="icb")
                nc.sync.dma_start(icb_s, init_cb)

                ps_h = initps.tile([P, 2 * HS], F32, tag="psh")
                for k in range(KF):
                    nc.tensor.matmul(ps_h, featT_s[:, k, :], ihw_s[:, k, :],
                                     start=(k == 0), stop=False)
                nc.tensor.matmul(ps_h, ones1, ihb_s, start=False, stop=True)
                ps_c = initps.tile([P, 2 * HS], F32, tag="psc")
                for k in range(KF):
                    nc.tensor.matmul(ps_c, featT_s[:, k, :], icw_s[:, k, :],
                                     start=(k == 0), stop=False)
                nc.tensor.matmul(ps_c, ones1, icb_s, start=False, stop=True)

                nc.vector.tensor_copy(c0_s, ps_c[:, 0:HS])
                nc.vector.tensor_copy(c1_s, ps_c[:, HS:2 * HS])
                h_init = initw.tile([P, 2 * HS], F16, tag="hinit")
                nc.vector.tensor_copy(h_init, ps_h)

                ib_in = drp.tile([P, 2 * HS], F16, tag="ibin")
                nc.scalar.dma_start(ib_in, h_init)
                ib_out = drp.tile([NCORES * P, 2 * HS], F16, tag="ibout")
                nc.gpsimd.collective_compute(
                    "AllGather", mybir.AluOpType.bypass, replica_groups=RG,
                    ins=[ib_in[:]], outs=[ib_out[:]],
                )
                h0T_cur = h0tp.tile([P, KH, B], F16, tag="h0T")
                nc.sync.dma_start_transpose(h0T_cur, ib_out[:, 0:HS])
                ysT_init = ystp.tile([P, KH, B], F16, tag="ysT")
                nc.sync.dma_start_transpose(ysT_init, ib_out[:, HS:2 * HS])

            # ------------- Phase D: recurrence + FC + inline X0 -----------
            with tc.tile_pool(name="act", bufs=3) as actp, \
                 tc.tile_pool(name="hstage", bufs=3) as hstage, \
                 tc.tile_pool(name="fco", bufs=4) as fcop, \
                 tc.tile_pool(name="g0ps", bufs=2, space="PSUM") as g0ps, \
                 tc.tile_pool(name="g1ps", bufs=2, space="PSUM") as g1ps, \
                 tc.tile_pool(name="fcps", bufs=2, space="PSUM") as fcps, \
                 tc.tile_pool(name="cps", bufs=1, space="PSUM") as cps, \
                 tc.tile_pool(name="warmps", bufs=1, space="PSUM") as warmps:

                def x0_compute(t):
                    ps = cps.tile([P, GS], F32, tag="cps")
                    for k in range(KE):
                        nc.tensor.matmul(
                            ps, embT_s[:, k, t * B:(t + 1) * B],
                            wih0_s[:, k, :], start=(k == 0), stop=False)
                    nc.tensor.matmul(ps, ones1, bsum0_s,
                                     start=False, stop=True)
                    nc.vector.tensor_copy(X0_s[:, t, :], ps)

                def lstm_tail(ps, c_s, sigtag, htag):
                    sig = actp.tile([P, 3 * HS], F32, tag=f"sig{sigtag}")
                    nc.scalar.activation(sig, ps[:, 0:3 * HS], SIG)
                    tg = actp.tile([P, HS], F32, tag=f"tg{sigtag}")
                    nc.scalar.activation(tg, ps[:, 3 * HS:4 * HS], TANH)
                    nc.vector.tensor_mul(c_s, sig[:, HS:2 * HS], c_s)
                    nc.vector.tensor_mul(tg, sig[:, 0:HS], tg)
                    nc.vector.tensor_add(c_s, c_s, tg)
                    tcn = actp.tile([P, HS], F32, tag=f"tc{sigtag}")
                    nc.scalar.activation(tcn, c_s, TANH)
                    h_sb = hstage.tile([P, HS], F16, tag=htag)
                    nc.vector.tensor_mul(h_sb, sig[:, 2 * HS:3 * HS], tcn)
                    return h_sb

                h0T_hist = {-1: h0T_cur}
                ysT_hist = {-1: ysT_init}

                # X0 prologue
                for t in range(min(X0AHEAD, T)):
                    x0_compute(t)

                # Super-step s: L1(s-1) first, then L0(s); ONE joint
                # AllGather carries [h0(s) | h1(s-1)] so only a single
                # collective is ever in flight (a queued second cc delays
                # ncfw's completion processing of the first by ~10us).
                for s in range(T + LAGF + 1):
                    t0, t1, t2 = s, s - LAG1, s - LAGF
                    tx = s + X0AHEAD
                    b_in = b_out = None
                    if t0 < T or (0 <= t1 < T):
                        b_in = drp.tile([P, 2 * HS], F16, tag="bin",
                                        name="bin")
                        b_out = drp.tile([NCORES * P, 2 * HS], F16,
                                         tag="bout", name="bout")

                    if 0 <= t1 < T:
                        ps1 = g1ps.tile([P, GS], F32, tag="g1")
                        # bias first: no dependency on gathered h, so it
                        # runs during the collective wait, off the chain
                        nc.tensor.matmul(ps1, ones1, bsum1_s,
                                         start=True, stop=False)
                        h0T_t1 = h0T_hist[t1]
                        for k in range(KH):
                            nc.tensor.matmul(ps1, h0T_t1[:, k, :],
                                             wih1_s[:, k, :],
                                             start=False, stop=False)
                        ysT_prev = ysT_hist[t1 - 1]
                        for k in range(KH):
                            nc.tensor.matmul(ps1, ysT_prev[:, k, :],
                                             whh1_s[:, k, :],
                                             start=False,
                                             stop=(k == KH - 1))
                        h1_sb = lstm_tail(ps1, c1_s, 1, "h1sb")
                        nc.scalar.dma_start(b_in[:, HS:2 * HS], h1_sb)
                        h0T_hist.pop(t1 - 1, None)
                        ysT_hist.pop(t1 - 4, None)

                    if t0 < T:
                        ps0 = g0ps.tile([P, GS], F32, tag="g0")
                        nc.tensor.matmul(ps0, id128, X0_s[:, t0, :],
                                         start=True, stop=False)
                        h0T_prev = h0T_hist[t0 - 1]
                        for k in range(KH):
                            nc.tensor.matmul(ps0, h0T_prev[:, k, :],
                                             whh0_s[:, k, :],
                                             start=False, stop=(k == KH - 1))
                        h0_sb = lstm_tail(ps0, c0_s, 0, "h0sb")
                        nc.scalar.dma_start(b_in[:, 0:HS], h0_sb)

                    if b_in is not None:
                        nc.gpsimd.collective_compute(
                            "AllGather", mybir.AluOpType.bypass,
                            replica_groups=RG,
                            ins=[b_in[:]], outs=[b_out[:]],
                        )

                    if 0 <= t2 < T:
                        yt = ysT_hist[t2]
                        for ccol, w in ((0, 512), (512, 512), (1024, VS - 1024)):
                            psf = fcps.tile([P, 512], F32, tag="fc")
                            for k in range(KH):
                                nc.tensor.matmul(psf[:, :w], yt[:, k, :],
                                                 fcw_s[:, k, ccol:ccol + w],
                                                 start=(k == 0),
                                                 stop=(k == KH - 1))
                            ot = fcop.tile([P, 512], F32, tag="fco")
                            nc.vector.tensor_add(ot[:, :w], psf[:, :w],
                                                 fcb_s[:, ccol:ccol + w])
                            nc.sync.dma_start(
                                out[t2 * B:(t2 + 1) * B, ccol:ccol + w],
                                ot[:, :w])

                    if tx < T:
                        x0_compute(tx)

                    # land the joint gather on the sync queue; h0T in two
                    # halves so the first whh0 k-tiles start sooner
                    if b_out is not None and t0 < T:
                        hT = h0tp.tile([P, KH, B], F16, tag="h0T")
                        nc.sync.dma_start_transpose(
                            hT[:, 0:KH // 2, :], b_out[0:NCORES * P // 2, 0:HS])
                        nc.sync.dma_start_transpose(
                            hT[:, KH // 2:KH, :],
                            b_out[NCORES * P // 2:NCORES * P, 0:HS])
                        h0T_hist[t0] = hT
                    if b_out is not None and 0 <= t1 < T:
                        yT = ystp.tile([P, KH, B], F16, tag="ysT")
                        nc.sync.dma_start_transpose(yT, b_out[:, HS:2 * HS])
                        ysT_hist[t1] = yT

    nc.finalize()
    return nc


def _get_compiled():
    if "nc" not in _cache:
        _cache["nc"] = _build_nc()
    return _cache["nc"]


def _prep_inputs(features, captions, embed_table, init_h_w, init_h_b,
                 init_c_w, init_c_b, w_ih0, w_hh0, b_ih0, b_hh0,
                 w_ih1, w_hh1, b_ih1, b_hh1, fc_w, fc_b):
    f32 = np.float32
    qorder = [0, 1, 3, 2]  # torch gate order i,f,g,o -> ours i,f,o,g

    def gate_cols(wmat, c):
        w = np.asarray(wmat, f32).reshape(4, H, -1)
        sl = w[:, c * HS:(c + 1) * HS, :][qorder]
        return np.ascontiguousarray(
            sl.reshape(4 * HS, -1).T).astype(np.float16)

    def gate_bias(b1, b2, c):
        sb = np.asarray(b1, f32) + np.asarray(b2, f32)
        sl = sb.reshape(4, H)[:, c * HS:(c + 1) * HS][qorder]
        return np.ascontiguousarray(sl.reshape(1, 4 * HS)).astype(np.float16)

    def init_wT(wmat, c):
        w = np.asarray(wmat, f32).reshape(H, L, F)
        blk = np.concatenate(
            [w[c * HS:(c + 1) * HS, 0, :], w[c * HS:(c + 1) * HS, 1, :]], 0)
        return np.ascontiguousarray(blk.T).astype(np.float16)

    def init_b(bvec, c):
        b = np.asarray(bvec, f32).reshape(H, L)
        blk = np.concatenate(
            [b[c * HS:(c + 1) * HS, 0], b[c * HS:(c + 1) * HS, 1]])
        return np.ascontiguousarray(blk.reshape(1, 2 * HS)).astype(np.float16)

    captions = np.asarray(captions)
    tbl = np.asarray(embed_table, f32)
    idx = captions.T.reshape(TB).astype(np.int64)
    emb = tbl[idx]
    emb[idx == 0] = 0.0
    embT_np = np.ascontiguousarray(emb.T).astype(np.float16)
    featT_np = np.ascontiguousarray(
        np.asarray(features, f32).T).astype(np.float16)

    fcw = np.asarray(fc_w, f32)
    fcb_np = np.asarray(fc_b, f32)

    in_maps = []
    for c in range(NCORES):
        vsl = slice(c * VS, (c + 1) * VS)
        in_maps.append({
            "featT": featT_np,
            "embT": embT_np,
            "init_hwT": init_wT(init_h_w, c),
            "init_cwT": init_wT(init_c_w, c),
            "init_hb": init_b(init_h_b, c),
            "init_cb": init_b(init_c_b, c),
            "wih0T": gate_cols(w_ih0, c),
            "whh0T": gate_cols(w_hh0, c),
            "wih1T": gate_cols(w_ih1, c),
            "whh1T": gate_cols(w_hh1, c),
            "bsum0": gate_bias(b_ih0, b_hh0, c),
            "bsum1": gate_bias(b_ih1, b_hh1, c),
            "fcwT": np.ascontiguousarray(fcw[vsl].T).astype(np.float16),
            "fcb": np.ascontiguousarray(
                np.broadcast_to(fcb_np[vsl], (P, VS))).astype(f32),
        })
    return in_maps


last_results = None


def kernel(**inputs) -> np.ndarray:
    global last_results
    nc = _get_compiled()
    in_maps = _prep_inputs(**inputs)
    res = run_bass_kernel_spmd(nc, in_maps, core_ids=list(range(NCORES)))
    last_results = res
    parts = [res.results[c]["out"].reshape(T, B, VS) for c in range(NCORES)]
    return np.concatenate(parts, axis=2)


# revision 4
# speedup vs baseline: 1.0560x; 1.0072x over previous
"""Trainium2 Bass kernel: 2-layer LSTM decoder, gate-sharded across 8 cores.

v3: one joint AllGather per step (the collective core serializes ccs at
~8.5us each, so two per step was the v2 bottleneck).

Super-step s:
  PE: L0(s) -> L1(s-1) -> FC(s-4) -> X0(s+5)
  AG(s) gathers [h0(s) | h1(s-2)] (h1 half staged during step s-1).
  Landing DMAs (transposing, DRAM->SBUF) own the sync queue; staging DMAs
  ride the scalar queue; the collective doorbell + FC output DMAs ride
  gpsimd. fp16 matmuls, fp32 PSUM/state.
"""

import numpy as np

import concourse.bass as bass
import concourse.mybir as mybir
from concourse import bacc
from concourse.bass_utils import run_bass_kernel_spmd
from concourse.masks import make_identity
from concourse.tile import TileContext

P = 128
NCORES = 8
B, T, F, E, H, L, V = 128, 32, 2048, 512, 1024, 2, 10000
G = 4 * H
TB = T * B
HS = H // NCORES           # 128
GS = 4 * HS                # 512
VS = V // NCORES           # 1250
KF, KE, KH = F // P, E // P, H // P
LAG1 = 1                   # L1 step lag
LAGF = 2                   # FC step lag (ysT lands ~1 step ahead of use)
X0AHEAD = 3                # X0 precompute lookahead inside the loop
F16 = mybir.dt.float16
F32 = mybir.dt.float32
RG = [list(range(NCORES))]

_cache = {}


def _build_nc():
    nc = bacc.Bacc("TRN2", target_bir_lowering=False, debug=False,
                   enable_asserts=False, num_devices=NCORES)

    def din(name, shape, dt=F16):
        return nc.dram_tensor(name, shape, dt, kind="ExternalInput").ap()

    featT = din("featT", [F, B])
    embT = din("embT", [E, TB])
    init_hwT = din("init_hwT", [F, 2 * HS])
    init_cwT = din("init_cwT", [F, 2 * HS])
    init_hb = din("init_hb", [1, 2 * HS])
    init_cb = din("init_cb", [1, 2 * HS])
    wih0T = din("wih0T", [E, GS])
    whh0T = din("whh0T", [H, GS])
    wih1T = din("wih1T", [H, GS])
    whh1T = din("whh1T", [H, GS])
    bsum0 = din("bsum0", [1, GS])
    bsum1 = din("bsum1", [1, GS])
    fcwT = din("fcwT", [H, VS])
    fcb = din("fcb", [P, VS], F32)

    out = nc.dram_tensor("out", [TB, VS], F32, kind="ExternalOutput").ap()

    featT_v = featT.rearrange("(k p) b -> p k b", p=P)
    embT_v = embT.rearrange("(k p) t -> p k t", p=P)
    ihw_v = init_hwT.rearrange("(k p) n -> p k n", p=P)
    icw_v = init_cwT.rearrange("(k p) n -> p k n", p=P)
    wih0_v = wih0T.rearrange("(k p) g -> p k g", p=P)
    whh0_v = whh0T.rearrange("(k p) g -> p k g", p=P)
    wih1_v = wih1T.rearrange("(k p) g -> p k g", p=P)
    whh1_v = whh1T.rearrange("(k p) g -> p k g", p=P)
    fcw_v = fcwT.rearrange("(k p) v -> p k v", p=P)

    SIG = mybir.ActivationFunctionType.Sigmoid
    TANH = mybir.ActivationFunctionType.Tanh

    with TileContext(nc) as tc:
        with tc.tile_pool(name="const", bufs=1) as constp, \
             tc.tile_pool(name="resw", bufs=1) as resw, \
             tc.tile_pool(name="x0", bufs=1) as x0p, \
             tc.tile_pool(name="state", bufs=1) as statep, \
             tc.tile_pool(name="h0t", bufs=3) as h0tp, \
             tc.tile_pool(name="yst", bufs=6) as ystp, \
             tc.tile_pool(name="dram", bufs=4, space="DRAM") as drp:

            id128 = constp.tile([P, P], F16)
            make_identity(nc, id128)
            ones1 = constp.tile([1, P], F16)
            nc.gpsimd.memset(ones1, 1.0)

            whh0_s = resw.tile([P, KH, GS], F16)
            nc.sync.dma_start(whh0_s, whh0_v)
            wih1_s = resw.tile([P, KH, GS], F16)
            nc.sync.dma_start(wih1_s, wih1_v)
            whh1_s = resw.tile([P, KH, GS], F16)
            nc.sync.dma_start(whh1_s, whh1_v)
            bsum1_s = resw.tile([1, GS], F16)
            nc.sync.dma_start(bsum1_s, bsum1)
            fcw_s = resw.tile([P, KH, VS], F16)
            nc.sync.dma_start(fcw_s, fcw_v)
            fcb_s = resw.tile([P, VS], F32)
            nc.sync.dma_start(fcb_s, fcb)
            embT_s = resw.tile([P, KE, TB], F16)
            nc.sync.dma_start(embT_s, embT_v)
            wih0_s = resw.tile([P, KE, GS], F16)
            nc.sync.dma_start(wih0_s, wih0_v)
            bsum0_s = resw.tile([1, GS], F16)
            nc.sync.dma_start(bsum0_s, bsum0)

            X0_s = x0p.tile([P, T, GS], F16)
            c0_s = statep.tile([P, HS], F32)
            c1_s = statep.tile([P, HS], F32)

            # ------------- Phase B: init h/c ------------------------------
            with tc.tile_pool(name="initw", bufs=1) as initw, \
                 tc.tile_pool(name="initps", bufs=1, space="PSUM") as initps:
                featT_s = initw.tile([P, KF, B], F16, tag="ft")
                nc.sync.dma_start(featT_s, featT_v)
                ihw_s = initw.tile([P, KF, 2 * HS], F16, tag="ihw")
                nc.sync.dma_start(ihw_s, ihw_v)
                icw_s = initw.tile([P, KF, 2 * HS], F16, tag="icw")
                nc.sync.dma_start(icw_s, icw_v)
                ihb_s = initw.tile([1, 2 * HS], F16, tag="ihb")
                nc.sync.dma_start(ihb_s, init_hb)
                icb_s = initw.tile([1, 2 * HS], F16, tag="icb")
                nc.sync.dma_start(icb_s, init_cb)

                ps_h = initps.tile([P, 2 * HS], F32, tag="psh")
                for k in range(KF):
                    nc.tensor.matmul(ps_h, featT_s[:, k, :], ihw_s[:, k, :],
                                     start=(k == 0), stop=False)
                nc.tensor.matmul(ps_h, ones1, ihb_s, start=False, stop=True)
                ps_c = initps.tile([P, 2 * HS], F32, tag="psc")
                for k in range(KF):
                    nc.tensor.matmul(ps_c, featT_s[:, k, :], icw_s[:, k, :],
                                     start=(k == 0), stop=False)
                nc.tensor.matmul(ps_c, ones1, icb_s, start=False, stop=True)

                nc.vector.tensor_copy(c0_s, ps_c[:, 0:HS])
                nc.vector.tensor_copy(c1_s, ps_c[:, HS:2 * HS])
                h_init = initw.tile([P, 2 * HS], F16, tag="hinit")
                nc.vector.tensor_copy(h_init, ps_h)

                ib_in = drp.tile([P, 2 * HS], F16, tag="ibin")
                nc.scalar.dma_start(ib_in, h_init)
                ib_out = drp.tile([NCORES * P, 2 * HS], F16, tag="ibout")
                nc.gpsimd.collective_compute(
                    "AllGather", mybir.AluOpType.bypass, replica_groups=RG,
                    ins=[ib_in[:]], outs=[ib_out[:]],
                )
                h0T_cur = h0tp.tile([P, KH, B], F16, tag="h0T")
                nc.sync.dma_start_transpose(h0T_cur, ib_out[:, 0:HS])
                ysT_init = ystp.tile([P, KH, B], F16, tag="ysT")
                nc.sync.dma_start_transpose(ysT_init, ib_out[:, HS:2 * HS])

            # ------------- Phase D: recurrence + FC + inline X0 -----------
            with tc.tile_pool(name="act", bufs=3) as actp, \
                 tc.tile_pool(name="hstage", bufs=3) as hstage, \
                 tc.tile_pool(name="fco", bufs=4) as fcop, \
                 tc.tile_pool(name="g0ps", bufs=2, space="PSUM") as g0ps, \
                 tc.tile_pool(name="g1ps", bufs=2, space="PSUM") as g1ps, \
                 tc.tile_pool(name="fcps", bufs=2, space="PSUM") as fcps, \
                 tc.tile_pool(name="cps", bufs=1, space="PSUM") as cps, \
                 tc.tile_pool(name="warmps", bufs=1, space="PSUM") as warmps:

                def x0_compute(t):
                    ps = cps.tile([P, GS], F32, tag="cps")
                    for k in range(KE):
                        nc.tensor.matmul(
                            ps, embT_s[:, k, t * B:(t + 1) * B],
                            wih0_s[:, k, :], start=(k == 0), stop=False)
                    nc.tensor.matmul(ps, ones1, bsum0_s,
                                     start=False, stop=True)
                    nc.vector.tensor_copy(X0_s[:, t, :], ps)

                def lstm_tail(ps, c_s, sigtag, htag):
                    # i,f sigmoids first so the c-state DVE chain starts
                    # sooner; o can wait until tanh(c) is ready anyway
                    sig = actp.tile([P, 3 * HS], F32, tag=f"sig{sigtag}")
                    nc.scalar.activation(sig[:, 0:2 * HS], ps[:, 0:2 * HS],
                                         SIG)
                    tg = actp.tile([P, HS], F32, tag=f"tg{sigtag}")
                    nc.scalar.activation(tg, ps[:, 3 * HS:4 * HS], TANH)
                    nc.scalar.activation(sig[:, 2 * HS:3 * HS],
                                         ps[:, 2 * HS:3 * HS], SIG)
                    nc.vector.tensor_mul(c_s, sig[:, HS:2 * HS], c_s)
                    nc.vector.tensor_mul(tg, sig[:, 0:HS], tg)
                    nc.vector.tensor_add(c_s, c_s, tg)
                    tcn = actp.tile([P, HS], F32, tag=f"tc{sigtag}")
                    nc.scalar.activation(tcn, c_s, TANH)
                    h_sb = hstage.tile([P, HS], F16, tag=htag)
                    nc.vector.tensor_mul(h_sb, sig[:, 2 * HS:3 * HS], tcn)
                    return h_sb

                h0T_hist = {-1: h0T_cur}
                ysT_hist = {-1: ysT_init}

                # X0 prologue
                for t in range(min(X0AHEAD, T)):
                    x0_compute(t)

                # Super-step s: L1(s-1) first, then L0(s); ONE joint
                # AllGather carries [h0(s) | h1(s-1)] so only a single
                # collective is ever in flight (a queued second cc delays
                # ncfw's completion processing of the first by ~10us).
                for s in range(T + LAGF + 1):
                    t0, t1, t2 = s, s - LAG1, s - LAGF
                    tx = s + X0AHEAD
                    b_in = b_out = None
                    if t0 < T or (0 <= t1 < T):
                        b_in = drp.tile([P, 2 * HS], F16, tag="bin",
                                        name="bin")
                        b_out = drp.tile([NCORES * P, 2 * HS], F16,
                                         tag="bout", name="bout")

                    if 0 <= t1 < T:
                        ps1 = g1ps.tile([P, GS], F32, tag="g1")
                        # bias first: no dependency on gathered h, so it
                        # runs during the collective wait, off the chain
                        nc.tensor.matmul(ps1, ones1, bsum1_s,
                                         start=True, stop=False)
                        h0T_t1 = h0T_hist[t1]
                        for k in range(KH):
                            nc.tensor.matmul(ps1, h0T_t1[:, k, :],
                                             wih1_s[:, k, :],
                                             start=False, stop=False)
                        ysT_prev = ysT_hist[t1 - 1]
                        for k in range(KH):
                            nc.tensor.matmul(ps1, ysT_prev[:, k, :],
                                             whh1_s[:, k, :],
                                             start=False,
                                             stop=(k == KH - 1))
                        h1_sb = lstm_tail(ps1, c1_s, 1, "h1sb")
                        nc.scalar.dma_start(b_in[:, HS:2 * HS], h1_sb)
                        h0T_hist.pop(t1 - 1, None)
                        ysT_hist.pop(t1 - 4, None)

                    if t0 < T:
                        ps0 = g0ps.tile([P, GS], F32, tag="g0")
                        nc.tensor.matmul(ps0, id128, X0_s[:, t0, :],
                                         start=True, stop=False)
                        h0T_prev = h0T_hist[t0 - 1]
                        for k in range(KH):
                            nc.tensor.matmul(ps0, h0T_prev[:, k, :],
                                             whh0_s[:, k, :],
                                             start=False, stop=(k == KH - 1))
                        h0_sb = lstm_tail(ps0, c0_s, 0, "h0sb")
                        nc.scalar.dma_start(b_in[:, 0:HS], h0_sb)

                    if b_in is not None:
                        nc.gpsimd.collective_compute(
                            "AllGather", mybir.AluOpType.bypass,
                            replica_groups=RG,
                            ins=[b_in[:]], outs=[b_out[:]],
                        )

                    if 0 <= t2 < T:
                        yt = ysT_hist[t2]
                        for ccol, w in ((0, 512), (512, 512), (1024, VS - 1024)):
                            psf = fcps.tile([P, 512], F32, tag="fc")
                            for k in range(KH):
                                nc.tensor.matmul(psf[:, :w], yt[:, k, :],
                                                 fcw_s[:, k, ccol:ccol + w],
                                                 start=(k == 0),
                                                 stop=(k == KH - 1))
                            ot = fcop.tile([P, 512], F32, tag="fco")
                            nc.vector.tensor_add(ot[:, :w], psf[:, :w],
                                                 fcb_s[:, ccol:ccol + w])
                            nc.sync.dma_start(
                                out[t2 * B:(t2 + 1) * B, ccol:ccol + w],
                                ot[:, :w])

                    if tx < T:
                        x0_compute(tx)

                    # land the joint gather on the sync queue; h0T in two
                    # halves so the first whh0 k-tiles start sooner
                    if b_out is not None and t0 < T:
                        hT = h0tp.tile([P, KH, B], F16, tag="h0T")
                        nc.sync.dma_start_transpose(
                            hT[:, 0:KH // 2, :], b_out[0:NCORES * P // 2, 0:HS])
                        nc.sync.dma_start_transpose(
                            hT[:, KH // 2:KH, :],
                            b_out[NCORES * P // 2:NCORES * P, 0:HS])
                        h0T_hist[t0] = hT
                    if b_out is not None and 0 <= t1 < T:
                        yT = ystp.tile([P, KH, B], F16, tag="ysT")
                        nc.sync.dma_start_transpose(yT, b_out[:, HS:2 * HS])
                        ysT_hist[t1] = yT

    nc.finalize()
    return nc


def _get_compiled():
    if "nc" not in _cache:
        _cache["nc"] = _build_nc()
    return _cache["nc"]


def _prep_inputs(features, captions, embed_table, init_h_w, init_h_b,
                 init_c_w, init_c_b, w_ih0, w_hh0, b_ih0, b_hh0,
                 w_ih1, w_hh1, b_ih1, b_hh1, fc_w, fc_b):
    f32 = np.float32
    qorder = [0, 1, 3, 2]  # torch gate order i,f,g,o -> ours i,f,o,g

    def gate_cols(wmat, c):
        w = np.asarray(wmat, f32).reshape(4, H, -1)
        sl = w[:, c * HS:(c + 1) * HS, :][qorder]
        return np.ascontiguousarray(
            sl.reshape(4 * HS, -1).T).astype(np.float16)

    def gate_bias(b1, b2, c):
        sb = np.asarray(b1, f32) + np.asarray(b2, f32)
        sl = sb.reshape(4, H)[:, c * HS:(c + 1) * HS][qorder]
        return np.ascontiguousarray(sl.reshape(1, 4 * HS)).astype(np.float16)

    def init_wT(wmat, c):
        w = np.asarray(wmat, f32).reshape(H, L, F)
        blk = np.concatenate(
            [w[c * HS:(c + 1) * HS, 0, :], w[c * HS:(c + 1) * HS, 1, :]], 0)
        return np.ascontiguousarray(blk.T).astype(np.float16)

    def init_b(bvec, c):
        b = np.asarray(bvec, f32).reshape(H, L)
        blk = np.concatenate(
            [b[c * HS:(c + 1) * HS, 0], b[c * HS:(c + 1) * HS, 1]])
        return np.ascontiguousarray(blk.reshape(1, 2 * HS)).astype(np.float16)

    captions = np.asarray(captions)
    tbl = np.asarray(embed_table, f32)
    idx = captions.T.reshape(TB).astype(np.int64)
    emb = tbl[idx]
    emb[idx == 0] = 0.0
    embT_np = np.ascontiguousarray(emb.T).astype(np.float16)
    featT_np = np.ascontiguousarray(
        np.asarray(features, f32).T).astype(np.float16)

    fcw = np.asarray(fc_w, f32)
    fcb_np = np.asarray(fc_b, f32)

    in_maps = []
    for c in range(NCORES):
        vsl = slice(c * VS, (c + 1) * VS)
        in_maps.append({
            "featT": featT_np,
            "embT": embT_np,
            "init_hwT": init_wT(init_h_w, c),
            "init_cwT": init_wT(init_c_w, c),
            "init_hb": init_b(init_h_b, c),
            "init_cb": init_b(init_c_b, c),
            "wih0T": gate_cols(w_ih0, c),
            "whh0T": gate_cols(w_hh0, c),
            "wih1T": gate_cols(w_ih1, c),
            "whh1T": gate_cols(w_hh1, c),
            "bsum0": gate_bias(b_ih0, b_hh0, c),
            "bsum1": gate_bias(b_ih1, b_hh1, c),
            "fcwT": np.ascontiguousarray(fcw[vsl].T).astype(np.float16),
            "fcb": np.ascontiguousarray(
                np.broadcast_to(fcb_np[vsl], (P, VS))).astype(f32),
        })
    return in_maps


last_results = None


def kernel(**inputs) -> np.ndarray:
    global last_results
    nc = _get_compiled()
    in_maps = _prep_inputs(**inputs)
    res = run_bass_kernel_spmd(nc, in_maps, core_ids=list(range(NCORES)))
    last_results = res
    parts = [res.results[c]["out"].reshape(T, B, VS) for c in range(NCORES)]
    return np.concatenate(parts, axis=2)
